# revision 1
# baseline (speedup 1.0000x reference)
"""BinaryLinear Trainium2 kernel: out = sign(x) @ sign(W).T

x: (4, 4096, 1024) f32, W: (1024, 1024) f32 -> out (4, 4096, 1024) f32.

Strategy (8 NeuronCores, data-parallel over flattened batch*seq):
  - Each core gets a [2048, 1024] row-shard of x and the full W.
  - Per core, per 128-row m-tile:
      DMA x tile [128m, 1024i] f32 -> ACT Sign (fp32 -> fp8e4, +-1/0 exact)
      -> xbar DMA transpose of the fp8 bytes viewed as u16 pairs, giving a
         [128p, 4c, 128m, 2b] layout where contraction index i = 256c + 2p + b
      -> 8 fp8 DoubleRow matmuls (K=256 each) accumulate [128m, 1024o] in PSUM
      -> DVE copy PSUM -> SBUF -> DMA out.
  - W is repacked once on the host (numpy, O(K*N)): sign -> fp8 -> the same
    (p, c, b) -> i contraction layout with o contiguous (N=512 moving
    operand); each core DMAs the packed 1MB tensor once.
  - Post-scheduling passes replace Tile's over-conservative DMA-lane waits
    with exact producer-based waits (see _fix_false_dma_coupling) and
    legalize wait counts to the ISA per-instruction limits
    (_legalize_dma_waits).

All arithmetic is exact: sign values are +-1/0 (exact in fp8e4) and the PE
accumulates in fp32, so results are exact integers <= 1024.
"""

import numpy as np

P = 128
K = 1024  # in_features
N = 1024  # out_features
N_CORES = 8
M_TOTAL = 4 * 4096
M_PER_CORE = M_TOTAL // N_CORES


def build_binary_linear(tc, out, x, w):
    """Emit the per-core Tile kernel.

    out: DRAM [M, 1024] f32, x: DRAM [M, 1024] f32, w: DRAM [1024, 1024] f32.
    """
    import concourse.mybir as mybir

    nc = tc.nc
    f32 = mybir.dt.float32
    fp8 = mybir.dt.float8e4
    u16 = mybir.dt.uint16
    Sign = mybir.ActivationFunctionType.Sign
    DR = mybir.MatmulPerfMode.DoubleRow

    M = x.shape[0]
    assert M % P == 0 and x.shape[1] == K and w.shape == (P, 8 * N)
    n_mtiles = M // P

    with (
        tc.tile_pool(name="wsb", bufs=1) as wpool,
        tc.tile_pool(name="xin", bufs=8) as xin_pool,
        tc.tile_pool(name="xt", bufs=6) as xt_pool,
        tc.tile_pool(name="osb", bufs=4) as out_pool,
        tc.tile_pool(name="ps", bufs=4, space="PSUM") as psum_pool,
    ):
        # ---- W: host-packed fp8 [128, 8*1024]; wT[p, (2c+b)*1024 + o]
        # = sign(W)[o, i] with i = 256c + 2p + b. One 1MB DMA. ----
        wT = wpool.tile([P, 8 * N], fp8)
        nc.gpsimd.dma_start(out=wT, in_=w)

        # view for matmul rhs slices: [p][jj][d][b][o]; cb = 4*jj + 2*d + b
        w5 = wT.rearrange("p (jj d b o) -> p jj d b o", jj=2, d=2, b=2)

        # ---- main loop: FUSE = 2 m-tiles (256 rows) per load/sign/transpose ----
        FUSE = 2
        n_fused = n_mtiles // FUSE
        xfs = []
        for ft in range(n_fused):
            r0 = ft * FUSE * P
            xf = xin_pool.tile([P, FUSE, K], f32, tag="xf32", name=f"xf_t{ft}")
            # rows r0..r0+256 as [p, a, i] with m = r0 + 128a + p
            nc.gpsimd.dma_start(
                out=xf,
                in_=x[r0 : r0 + FUSE * P].rearrange("(a p) i -> p a i", p=P),
            )
            xfs.append(xf)
        for ft in range(n_fused):
            r0 = ft * FUSE * P
            xf = xfs[ft]
            x8 = xin_pool.tile([P, FUSE, K], fp8, tag="xfp8")
            nc.scalar.activation(out=x8, in_=xf, func=Sign)
            xt2 = xt_pool.tile([P, FUSE * K], fp8, tag="xt2")
            # in [128m, 1024 u16-units (a, i-pair)] -> out [128p, 8, 128m]
            # chunk index cc = 4a + c; within: i = 256c + 2p + b
            nc.sync.dma_start_transpose(
                out=xt2.bitcast(u16).rearrange("p (cc m) -> p cc m", cc=4 * FUSE),
                in_=x8.bitcast(u16).rearrange("p a u -> p (a u)"),
            )
            x5 = xt2.rearrange("p (a c m b) -> p a c m b", a=FUSE, c=4, b=2)

            osb = out_pool.tile([P, FUSE, N], f32, tag="osb")
            for a in range(FUSE):
                ps = [
                    psum_pool.tile([P, 512], f32, tag="ps0", name="ps0"),
                    psum_pool.tile([P, 512], f32, tag="ps1", name="ps1"),
                ]
                for idx, (j, b) in enumerate(((0, 0), (0, 1), (1, 0), (1, 1))):
                    lhsT = x5[:, a, 2 * j : 2 * j + 2, :, b]  # [p][c:2][m:128]
                    for h in range(2):
                        nc.tensor.matmul(
                            ps[h],
                            lhsT=lhsT,
                            rhs=w5[:, j, :, b, h * 512 : (h + 1) * 512],
                            start=(idx == 0),
                            stop=(idx == 3),
                            perf_mode=DR,
                        )
                for h in range(2):
                    nc.vector.tensor_copy(
                        out=osb[:, a, h * 512 : (h + 1) * 512], in_=ps[h]
                    )
            nc.gpsimd.dma_start(
                out=out[r0 : r0 + FUSE * P].rearrange("(a p) i -> p a i", p=P),
                in_=osb,
            )


def _fix_false_dma_coupling(nc, n_x_bufs, n_w_bufs=4):
    """Replace Tile's over-conservative / lane-aliased DMA waits on the whole
    load->sign->transpose front-end with exact producer-based waits computed
    from the scheduled stream. Tile's sem pass expresses old slot-WAR deps via
    "dominating" recent (sometimes *future*) DMA-lane events, which couples
    the pipeline into lock-step. We know the true dependency structure:

      wf_load[t]  <- w_sign[t - n_w_bufs]            (xf-slot WAR)
      w_sign[t]   <- wf_load[t], w_xpose[t - n_w_bufs]
      w_xpose[t]  <- w_sign[t], w_copy[t - n_w_bufs]
      w_copy[t]   <- w_xpose[t]
      x_load[ft]  <- x_sign[ft - n_x_bufs]  (ft >= bufs)
                     else w_sign[min(ft, last)]      (stagger behind W prep)
      x_sign[ft]  <- x_load[ft], x_xpose[ft - n_x_bufs]
      x_xpose[ft] <- x_sign[ft] (+ keep Tile's PE wait = xt2-slot WAR)

    Waits are emitted as (producer's update-sem >= cumulative value after it).
    Soundness is validated by CoreSim's race detector in the dev harness.
    """
    import concourse.mybir as mybir

    insts = []
    for f in nc.m.functions:
        for bb in f.blocks:
            insts.extend(bb.instructions)

    # cumulative sem value after each instruction's update
    cum = {}
    upd_after = {}  # inst name -> (sem_name, sem_id, cum_value_after)
    lane_order = {}  # inst name -> SyncWait enforcing same-lane completion order
    seqs = {k: [] for k in ("wf", "w8", "wt2", "wT", "xf", "x8", "xt2", "osb", "out")}
    for ins in insts:
        si = getattr(ins, "sync_info", None)
        if si is None:
            continue
        for u in si.on_update or []:
            prev = cum.get(u.ant_name, 0)
            if prev > 0 and (
                u.ant_name.startswith("DMAHW") or u.ant_name.startswith("DMASW")
            ):
                # DMA completions on one lane sem are not ordered by the HW;
                # the n-th updater must wait for the (n-1)-th's value or a
                # consumer's >= wait could be satisfied by the wrong DMA.
                lane_order[ins.name] = mybir.SyncWait(
                    sync_type="semaphore",
                    id=u.id,
                    ant_name=u.ant_name,
                    wait_mode="sem-ge-imm",
                    wait_value=prev,
                )
            cum[u.ant_name] = prev + u.update_value
            upd_after[ins.name] = (u.ant_name, u.id, cum[u.ant_name])
        memref = str(getattr(ins.outs[0], "memref", "")) if ins.outs else ""
        tn = type(ins).__name__
        for pref, want_tn in (
            ("wf", "InstDMACopy"),
            ("w8", "InstActivation"),
            ("wt2", "InstDmaTransposeAnt"),
            ("wT", "InstTensorCopy"),
            ("xf", "InstDMACopy"),
            ("x8", "InstActivation"),
            ("xt2", "InstDmaTransposeAnt"),
            ("osb", "InstTensorCopy"),
            ("out", "InstDMACopy"),
        ):
            if tn == want_tn and memref.startswith(pref):
                seqs[pref].append(ins)
                break

    def wait_on(producer_ins):
        sem_name, sem_id, v = upd_after[producer_ins.name]
        return mybir.SyncWait(
            sync_type="semaphore",
            id=sem_id,
            ant_name=sem_name,
            wait_mode="sem-ge-imm",
            wait_value=v,
        )

    def set_waits(ins, producers, extra=()):
        si = ins.sync_info
        waits = [wait_on(p) for p in producers if p is not None] + list(extra)
        lo = lane_order.get(ins.name)
        if lo is not None:
            waits.append(lo)
        ins.sync_info = mybir.SyncInfo(
            on_wait=waits, on_update=list(si.on_update or [])
        )

    def back(seq, i, k):
        return seq[i - k] if i >= k else None

    BW, BX = n_w_bufs, n_x_bufs
    for t, ins in enumerate(seqs["wf"]):
        set_waits(ins, [back(seqs["w8"], t, BW)])
    for t, ins in enumerate(seqs["w8"]):
        set_waits(ins, [seqs["wf"][t], back(seqs["wt2"], t, BW)])
    for t, ins in enumerate(seqs["wt2"]):
        set_waits(ins, [seqs["w8"][t], back(seqs["wT"], t, BW)])
    for t, ins in enumerate(seqs["wT"]):
        set_waits(ins, [seqs["wt2"][t]])
    for ft, ins in enumerate(seqs["xf"]):
        if ft >= BX:
            dep = seqs["x8"][ft - BX]
        else:
            dep = None
        set_waits(ins, [dep])
    for ft, ins in enumerate(seqs["x8"]):
        set_waits(ins, [seqs["xf"][ft], back(seqs["xt2"], ft, BX)])
    n_cp = len(seqs["osb"]) // max(len(seqs["out"]), 1)
    for ft, ins in enumerate(seqs["out"]):
        set_waits(ins, [seqs["osb"][(ft + 1) * n_cp - 1]])
    for ft, ins in enumerate(seqs["xt2"]):
        # keep Tile's non-lane waits (PE = xt2-slot WAR), add the sign RAW
        keep = [
            w
            for w in (ins.sync_info.on_wait or [])
            if not (w.ant_name.startswith("DMAHW") or w.ant_name.startswith("DMASW")
                    or w.ant_name.startswith("Activation"))
        ]
        set_waits(ins, [seqs["x8"][ft]], extra=keep)
    return {k: len(v) for k, v in seqs.items()}


def _legalize_dma_waits(nc):
    """Walrus caps in-struct sem waits: DMA_DIRECT2D_XPOSE takes 1, DMACopy 2.

    Tile's sem assignment is not transitively minimal and can emit 2-4 waits
    on DMA instructions. Hoist the excess into InstEventSemaphore wait-only
    instructions inserted just before the DMA on its triggering queue. This
    is sound: the queue executes the hoisted wait strictly before pushing the
    DMA descriptor, so the dependency is enforced (more conservatively) at
    trigger time instead of ring-pop time.
    """
    import concourse.mybir as mybir

    limits = {
        "InstDmaTransposeAnt": 1,
        "InstDMACopy": 1,
        "InstTensorCopy": 1,
        "InstActivation": 1,
        "InstMatmult": 1,
        "InstLdweights": 1,
        "InstMemset": 1,
        "InstTensorTensor": 1,
        "InstDrain": 1,
    }
    n_hoisted = 0
    for f in nc.m.functions:
        for bb in f.blocks:
            new_list = []
            for ins in bb.instructions:
                lim = limits.get(type(ins).__name__)
                si = getattr(ins, "sync_info", None)
                waits = list(si.on_wait) if si is not None and si.on_wait else []
                if lim is not None and len(waits) > lim:
                    # keep data-producer (engine-sem) waits in-struct first,
                    # then the freshest DMA-lane waits; hoist the rest
                    def keep_rank(w):
                        is_lane = w.ant_name.startswith(
                            "DMAHW"
                        ) or w.ant_name.startswith("DMASW")
                        return (1 if is_lane else 0, -w.wait_value)

                    waits_sorted = sorted(waits, key=keep_rank)
                    keep, hoist = waits_sorted[:lim], waits_sorted[lim:]
                    for ci in range(0, len(hoist), 2):
                        chunk = hoist[ci : ci + 2]
                        ev = mybir.InstEventSemaphore(
                            name=f"{ins.name}-prewait{ci // 2}",
                            engine=ins.engine,
                            ins=[],
                            outs=[],
                            sync_info=mybir.SyncInfo(on_wait=chunk, on_update=[]),
                        )
                        nc.inst_map[ev.name] = ev
                        new_list.append(ev)
                        n_hoisted += len(chunk)
                    ins.sync_info = mybir.SyncInfo(
                        on_wait=keep, on_update=list(si.on_update or [])
                    )
                new_list.append(ins)
            bb.instructions[:] = new_list
    return n_hoisted


def _build_nc(m_per_core):
    import concourse.bass as bass
    import concourse.mybir as mybir
    from concourse import tile

    nc = bass.Bass("TRN2", target_bir_lowering=False, num_swdge_queues=4)
    x_d = nc.dram_tensor("x", [m_per_core, K], mybir.dt.float32, kind="ExternalInput")
    w_d = nc.dram_tensor("W", [P, 8 * N], mybir.dt.float8e4, kind="ExternalInput")
    out_d = nc.dram_tensor(
        "out", [m_per_core, N], mybir.dt.float32, kind="ExternalOutput"
    )
    with tile.TileContext(nc) as tc:
        build_binary_linear(tc, out_d.ap(), x_d.ap(), w_d.ap())
    _fix_false_dma_coupling(nc, n_x_bufs=8, n_w_bufs=8)
    _legalize_dma_waits(nc)
    return nc


_cached = {}


def _get_nc(m_per_core):
    if m_per_core not in _cached:
        _cached[m_per_core] = _build_nc(m_per_core)
    return _cached[m_per_core]


def kernel(x, W, _trace=False):
    from concourse import bass_utils

    import ml_dtypes

    xf = np.ascontiguousarray(np.asarray(x, dtype=np.float32).reshape(M_TOTAL, K))
    # pack sign(W) into the fp8 on-chip layout: wp[p, (c,b), o] = sign(W)[o, i],
    # i = 256c + 2p + b  (weight repacking, done once on host)
    sT = np.sign(np.asarray(W, dtype=np.float32)).T.astype(ml_dtypes.float8_e4m3)
    wp = np.ascontiguousarray(
        sT.reshape(4, P, 2, N).transpose(1, 0, 2, 3).reshape(P, 8 * N)
    )
    in_maps = [
        {"x": xf[i * M_PER_CORE : (i + 1) * M_PER_CORE], "W": wp}
        for i in range(N_CORES)
    ]
    nc = _get_nc(M_PER_CORE)
    res = bass_utils.run_bass_kernel_spmd(
        nc, in_maps, core_ids=list(range(N_CORES)), trace=_trace
    )
    out = np.concatenate([r["out"] for r in res.results], axis=0)
    out = out.reshape(4, 4096, N).astype(np.float32)
    if _trace:
        kernel.last_results = res
    return out



# revision 4
# speedup vs baseline: 1.1621x; 1.1621x over previous
"""BinaryLinear Trainium2 kernel: out = sign(x) @ sign(W).T

x: (4, 4096, 1024) f32, W: (1024, 1024) f32 -> out (4, 4096, 1024) f32.

Strategy (8 NeuronCores, data-parallel over flattened batch*seq):
  - Each core gets a [2048, 1024] row-shard of x and the full W.
  - x is re-laid-out on the host (pure permutation, no arithmetic) so the
    contraction index i lands on SBUF partitions directly: per core the DRAM
    tensor is [8 chunks * 128 p, (4 j, 2 c, 2 t, 128 u)] f32 with
    i = 256 j + 128 c + p and row m = 256 ch + 2 u + t. This removes the
    on-chip transpose entirely and loads with 8 KiB-per-partition contiguous
    descriptors.
  - Per chunk (256 rows): DMA 1 MiB -> ACT Sign (f32 -> fp8e4, +-1/0 exact)
    -> 16 fp8 DoubleRow matmuls (K=256 each) accumulating [128 m, 512 o]
    PSUM tiles -> DVE copy PSUM -> SBUF as float16 -> 0.5 MiB DMA out.
  - Outputs are exact integers |v| <= 1024, representable exactly in fp16,
    so stores are half-width; the host upcasts to f32. The evens/odds row
    interleave (t bit) makes each store descriptor cover 2 adjacent DRAM
    rows = 4 KiB.
  - W is repacked once on the host: wq[p, (j, c, o)] = sign(W)[o, i] fp8;
    each core DMAs the packed 1 MiB tensor once.

All arithmetic is exact: sign values are +-1/0 (exact in fp8e4), the PE
accumulates in fp32, and |out| <= 1024 is exact in fp16.
"""

import numpy as np

P = 128
K = 1024  # in_features
N = 1024  # out_features
N_CORES = 8
M_TOTAL = 4 * 4096
M_PER_CORE = M_TOTAL // N_CORES
MC = 256  # rows per chunk
N_CH = M_PER_CORE // MC


def build_binary_linear(tc, out, x, w):
    """Emit the per-core Tile kernel.

    out: DRAM [M_PER_CORE, N] f16, x: DRAM [N_CH*P, 8*MC] f32 (host-packed),
    w: DRAM [P, 8*N] fp8 (host-packed).
    """
    import concourse.mybir as mybir

    nc = tc.nc
    f32 = mybir.dt.float32
    f16 = mybir.dt.float16
    fp8 = mybir.dt.float8e4
    Sign = mybir.ActivationFunctionType.Sign
    DR = mybir.MatmulPerfMode.DoubleRow

    with (
        tc.tile_pool(name="wsb", bufs=1) as wpool,
        tc.tile_pool(name="xin", bufs=4) as xin_pool,
        tc.tile_pool(name="x8p", bufs=3) as x8_pool,
        tc.tile_pool(name="osb", bufs=4) as out_pool,
        tc.tile_pool(name="ps", bufs=2, space="PSUM") as psum_pool,
    ):
        # ---- W: host-packed fp8 [128, 8*1024]; wq[p, (j, c, o)]
        # = sign(W)[o, i] with i = 256j + 128c + p. One 1MB DMA. ----
        wT = wpool.tile([P, 8 * N], fp8)
        nc.gpsimd.dma_start(out=wT, in_=w)
        w4 = wT.rearrange("p (j c o) -> p j c o", j=4, c=2)

        for ch in range(N_CH):
            xf = xin_pool.tile([P, 8 * MC], f32, tag="xf", name=f"xf{ch}")
            nc.gpsimd.dma_start(out=xf, in_=x[ch * P : (ch + 1) * P, :])
            x8 = x8_pool.tile([P, 8 * MC], fp8, tag="x8", name=f"x8{ch}")
            nc.scalar.activation(out=x8, in_=xf, func=Sign)
            x84 = x8.rearrange("p (j c m) -> p j c m", j=4, c=2)

            osb = out_pool.tile([P, 2 * N], f16, tag="osb", name=f"osb{ch}")
            osb2 = osb.rearrange("p (b o) -> p b o", b=2)
            for t in range(MC // P):
                ps = [
                    psum_pool.tile([P, 512], f32, tag=f"ps{t}{h}", name=f"ps{t}{h}")
                    for h in range(2)
                ]
                for j in range(4):
                    lhsT = x84[:, j, :, t * P : (t + 1) * P]
                    for h in range(2):
                        nc.tensor.matmul(
                            ps[h],
                            lhsT=lhsT,
                            rhs=w4[:, j, :, h * 512 : (h + 1) * 512],
                            start=(j == 0),
                            stop=(j == 3),
                            perf_mode=DR,
                        )
                for h in range(2):
                    nc.vector.tensor_copy(
                        out=osb2[:, t, h * 512 : (h + 1) * 512], in_=ps[h]
                    )
            # out rows ch*256 + 2p + t  <-  osb[p, t, o]
            nc.gpsimd.dma_start(
                out=out[ch * MC : (ch + 1) * MC].rearrange("(p b) o -> p (b o)", b=2),
                in_=osb,
            )


def _legalize_dma_waits(nc):
    """Walrus caps in-struct sem waits (DMA_DIRECT2D takes 1, DMACopy 2).

    Tile's sem assignment is not transitively minimal and can emit 2-4 waits
    on DMA instructions. Hoist the excess into InstEventSemaphore wait-only
    instructions inserted just before the DMA on its triggering queue. This
    is sound: the queue executes the hoisted wait strictly before pushing the
    DMA descriptor, so the dependency is enforced (more conservatively) at
    trigger time instead of ring-pop time.
    """
    import concourse.mybir as mybir

    limits = {
        "InstDmaTransposeAnt": 1,
        "InstDMACopy": 1,
        "InstTensorCopy": 1,
        "InstActivation": 1,
        "InstMatmult": 1,
        "InstLdweights": 1,
        "InstMemset": 1,
        "InstTensorTensor": 1,
        "InstDrain": 1,
    }
    n_hoisted = 0
    for f in nc.m.functions:
        for bb in f.blocks:
            new_list = []
            for ins in bb.instructions:
                lim = limits.get(type(ins).__name__)
                si = getattr(ins, "sync_info", None)
                waits = list(si.on_wait) if si is not None and si.on_wait else []
                if lim is not None and len(waits) > lim:
                    # keep data-producer (engine-sem) waits in-struct first,
                    # then the freshest DMA-lane waits; hoist the rest
                    def keep_rank(w):
                        is_lane = w.ant_name.startswith(
                            "DMAHW"
                        ) or w.ant_name.startswith("DMASW")
                        return (1 if is_lane else 0, -w.wait_value)

                    waits_sorted = sorted(waits, key=keep_rank)
                    keep, hoist = waits_sorted[:lim], waits_sorted[lim:]
                    for ci in range(0, len(hoist), 2):
                        chunk = hoist[ci : ci + 2]
                        ev = mybir.InstEventSemaphore(
                            name=f"{ins.name}-prewait{ci // 2}",
                            engine=ins.engine,
                            ins=[],
                            outs=[],
                            sync_info=mybir.SyncInfo(on_wait=chunk, on_update=[]),
                        )
                        nc.inst_map[ev.name] = ev
                        new_list.append(ev)
                        n_hoisted += len(chunk)
                    ins.sync_info = mybir.SyncInfo(
                        on_wait=keep, on_update=list(si.on_update or [])
                    )
                new_list.append(ins)
            bb.instructions[:] = new_list
    return n_hoisted


def _build_nc():
    import concourse.bass as bass
    import concourse.mybir as mybir
    from concourse import tile

    nc = bass.Bass("TRN2", target_bir_lowering=False, num_swdge_queues=4)
    x_d = nc.dram_tensor(
        "x", [N_CH * P, 8 * MC], mybir.dt.float32, kind="ExternalInput"
    )
    w_d = nc.dram_tensor("W", [P, 8 * N], mybir.dt.float8e4, kind="ExternalInput")
    out_d = nc.dram_tensor(
        "out", [M_PER_CORE, N], mybir.dt.float16, kind="ExternalOutput"
    )
    with tile.TileContext(nc) as tc:
        build_binary_linear(tc, out_d.ap(), x_d.ap(), w_d.ap())
    _legalize_dma_waits(nc)
    return nc


_cached = {}


def _get_nc():
    if "nc" not in _cached:
        _cached["nc"] = _build_nc()
    return _cached["nc"]


def kernel(x, W, _trace=False):
    from concourse import bass_utils

    import ml_dtypes

    xf = np.asarray(x, dtype=np.float32).reshape(M_TOTAL, K)
    # host re-layout (pure permutation): per core [ (ch, p), (j, c, t, u) ]
    # with m = 2048*core + 256*ch + 2u + t and i = 256j + 128c + p
    T = xf.reshape(N_CORES, N_CH, P, 2, 4, 2, P)  # (core, ch, u, t, j, c, p)
    xh = np.ascontiguousarray(T.transpose(0, 1, 6, 4, 5, 3, 2)).reshape(
        N_CORES, N_CH * P, 8 * MC
    )
    # pack sign(W) fp8: wq[p, (j, c, o)] = sign(W)[o, 256j + 128c + p]
    sT = np.sign(np.asarray(W, dtype=np.float32)).T.astype(ml_dtypes.float8_e4m3)
    wq = np.ascontiguousarray(
        sT.reshape(4, 2, P, N).transpose(2, 0, 1, 3)
    ).reshape(P, 8 * N)
    in_maps = [{"x": xh[i], "W": wq} for i in range(N_CORES)]
    nc = _get_nc()
    res = bass_utils.run_bass_kernel_spmd(
        nc, in_maps, core_ids=list(range(N_CORES)), trace=_trace
    )
    out = np.concatenate([r["out"] for r in res.results], axis=0)
    out = out.astype(np.float32).reshape(4, 4096, N)
    if _trace:
        kernel.last_results = res
    return out
